# revision 8
# baseline (speedup 1.0000x reference)
"""Trainium2 Bass kernel for nn_DigitConvolutionalModel (dense_cnn).

Model: y = relu(conv3x3(x) @ w1.T + b1) @ w2.T + b2, x: [65536, 784] f32.

Strategy:
  * The 3x3 valid conv (784 -> 676) and FC1 (676 -> 128) are both linear,
    so they fuse on the host into one effective weight W1e = w1 @ C with
    shape [128, 784] (C is the sparse conv operator). The device then runs
    a pure GEMM pipeline: y = relu(x @ W1e.T + b1) @ w2.T + b2.
  * Pure data parallel over 8 NeuronCores: each core gets 8192 rows of x.
  * Per-core shards are pre-transposed on the host to xT [784, 8192] so the
    contraction dim lands on SBUF partitions with fully contiguous DMA loads
    (no on-chip transposes; DMA x-bar transpose is 2-byte-dtype only).
  * On device, per 512-column batch block: 7 accumulating matmuls
    (K=112 each) into PSUM [128, 512], fused bias+ReLU on the scalar engine,
    one matmul [10, 512] for FC2, bias add on the vector engine, store.
    Output comes back as yT [10, 8192] per core; host transposes/concats.
"""

import os

import numpy as np

import concourse.bass as bass
import concourse.mybir as mybir
import concourse.tile as tile
from concourse.bass import ts
from concourse.bass_utils import run_bass_kernel_spmd

H = W = 28
KH = KW = 3
CIN = H * W  # 784
HID = 128
OUT = 10
B_TOTAL = 65536
NCORES = 8
BS = B_TOTAL // NCORES  # 8192 rows per core
NB = 512  # batch columns per block (fp32 PSUM bank limit)
NBLK = BS // NB  # 16
KCH = 112  # contraction chunk (112 * 7 = 784)
KC = CIN // KCH  # 7

# float32r: single-pass reduced-precision fp32 matmul mode (1 cycle/row at
# N>=256 vs 4 cycles/row for exact fp32). Flip with BASS_FP32R=0.
USE_F32R = os.environ.get("BASS_FP32R", "1") != "0"


def _build_nc():
    f32 = mybir.dt.float32
    # Matmul-operand dtype: float32r tells the PE to run its single-pass
    # reduced-precision fp32 mode. The BIR verifier requires fp32r matmul
    # inputs to be *typed* fp32r at their producer, so the DRAM tensors and
    # SBUF tiles feeding matmuls carry this dtype (numpy side is still f32;
    # same 4-byte layout).
    mdt = mybir.dt.float32r if USE_F32R else f32
    nc = bass.Bass()
    xt = nc.dram_tensor("xt", [CIN, BS], mdt, kind="ExternalInput")
    w1t = nc.dram_tensor("w1t", [CIN, HID], mdt, kind="ExternalInput")
    b1d = nc.dram_tensor("b1d", [HID, 1], f32, kind="ExternalInput")
    w2t = nc.dram_tensor("w2t", [HID, OUT], mdt, kind="ExternalInput")
    b2d = nc.dram_tensor("b2d", [OUT, 1], f32, kind="ExternalInput")
    yt = nc.dram_tensor("yt", [OUT, BS], f32, kind="ExternalOutput")

    with tile.TileContext(nc) as tc:
        with (
            tc.tile_pool(name="consts", bufs=1) as consts,
            tc.tile_pool(name="xin", bufs=3) as xin,
            # bufs=NBLK: slots never recycle, so no WAR waits ever land on
            # the relu / bias-add instructions (walrus: max 1 wait/inst).
            tc.tile_pool(name="hpool", bufs=NBLK) as hpool,
            tc.tile_pool(name="opool", bufs=NBLK) as opool,
            tc.tile_pool(name="ps1", bufs=2, space="PSUM") as ps1p,
            tc.tile_pool(name="ps2", bufs=2, space="PSUM") as ps2p,
        ):
            # FC1 weight, chunked [k, chunk, hid]: partition k in 0..111,
            # chunk c selects rows c*112..c*112+111 of w1t.
            w1_t = consts.tile([KCH, KC, HID], mdt)
            nc.sync.dma_start(w1_t[:], w1t.rearrange("(c k) m -> k c m", k=KCH))
            b1_t = consts.tile([HID, 1], f32)
            nc.sync.dma_start(b1_t[:], b1d[:])
            w2_t = consts.tile([HID, OUT], mdt)
            nc.sync.dma_start(w2_t[:], w2t[:])
            b2_t = consts.tile([OUT, 1], f32)
            nc.sync.dma_start(b2_t[:], b2d[:])

            # Pre-touch the bias tiles on their consumer engines so the main
            # relu (ACT) / bias-add (DVE) instructions don't need a second
            # sync-wait for the bias DMA (walrus allows 1 wait per inst).
            b1_probe = consts.tile([1, 1], f32)
            nc.scalar.copy(b1_probe[:], b1_t[0:1, 0:1])
            b2_probe = consts.tile([1, 1], f32)
            nc.vector.tensor_copy(b2_probe[:], b2_t[0:1, 0:1])

            # fp32/fp32r matmuls self-load their weights (no separate
            # LDWEIGHTS), so every semaphore wait lands on the Matmult
            # itself — and walrus only allows one sync-wait there. Tiny
            # dummy bf16 ldweights "probes" reading 1 element of a tile
            # absorb the cross-engine waits into the PE's in-order stream
            # before each matmul group. The loaded garbage weight is
            # irrelevant (the real matmuls self-load).
            def probe(ap):
                nc.tensor.ldweights(ap[0:1, 0:1].bitcast(mybir.dt.bfloat16))

            xt_r = xt.rearrange("(c k) n -> k c n", k=KCH)
            for i in range(NBLK):
                x_t = xin.tile([KCH, KC, NB], mdt)
                nc.sync.dma_start(x_t[:], xt_r[:, :, ts(i, NB)])

                if i == 0:
                    probe(w1_t[:, 0, :])
                probe(x_t[:, 0, :])
                ps = ps1p.tile([HID, NB], f32)
                for c in range(KC):
                    nc.tensor.matmul(
                        ps[:],
                        w1_t[:, c, :],
                        x_t[:, c, :],
                        start=(c == 0),
                        stop=(c == KC - 1),
                    )

                h = hpool.tile([HID, NB], mdt)
                nc.scalar.activation(
                    h[:], ps[:], mybir.ActivationFunctionType.Relu, bias=b1_t[:]
                )

                if i == 0:
                    probe(w2_t[:])
                probe(h[:])
                ps2 = ps2p.tile([OUT, NB], f32)
                nc.tensor.matmul(
                    ps2[:], w2_t[:], h[:], start=True, stop=True
                )

                o = opool.tile([OUT, NB], f32)
                nc.vector.tensor_scalar_add(o[:], ps2[:], b2_t[:])
                nc.scalar.dma_start(yt[:, ts(i, NB)], o[:])

    # This walrus build allows one sync-wait per instruction; Tile emits
    # multi-waits (e.g. slot-recycle WAW + readers-release on DMAs). Split
    # them into event-semaphore chains, same as bacc.compile() does.
    import bass_rust

    bass_rust.generate_event_semaphores(nc)
    return nc


def _fuse_conv_fc1(conv_w, w1):
    """W1e = w1 @ C where C is the 3x3 valid-conv operator [676, 784]."""
    cw = np.asarray(conv_w, np.float64).reshape(KH, KW)
    w1_r = np.asarray(w1, np.float64).reshape(HID, H - KH + 1, W - KW + 1)
    w1e = np.zeros((HID, H, W), np.float64)
    for a in range(KH):
        for b in range(KW):
            w1e[:, a : a + H - KH + 1, b : b + W - KW + 1] += w1_r * cw[a, b]
    return w1e.reshape(HID, CIN).astype(np.float32)


def _run(x, conv_w, w1, b1, w2, b2, trace=False):
    x = np.asarray(x, np.float32)
    w1e_t = np.ascontiguousarray(_fuse_conv_fc1(conv_w, w1).T)  # [784, 128]
    w2t = np.ascontiguousarray(np.asarray(w2, np.float32).T)  # [128, 10]
    b1c = np.ascontiguousarray(np.asarray(b1, np.float32).reshape(HID, 1))
    b2c = np.ascontiguousarray(np.asarray(b2, np.float32).reshape(OUT, 1))

    nc = _build_nc()
    in_maps = []
    for c in range(NCORES):
        xs = np.ascontiguousarray(x[c * BS : (c + 1) * BS].T)  # [784, 8192]
        in_maps.append(
            {"xt": xs, "w1t": w1e_t, "b1d": b1c, "w2t": w2t, "b2d": b2c}
        )
    res = run_bass_kernel_spmd(nc, in_maps, list(range(NCORES)), trace=trace)

    y = np.empty((B_TOTAL, OUT), np.float32)
    for c, r in enumerate(res.results):
        y[c * BS : (c + 1) * BS] = r["yt"].T
    return y, res


def kernel(x, conv_w, w1, b1, w2, b2):
    y, _ = _run(x, conv_w, w1, b1, w2, b2)
    return y
